# revision 17
# baseline (speedup 1.0000x reference)
"""GCN link-prediction (3-layer GCNConv encode + dot-product decode) on 8 trn2
NeuronCores via Bass/Tile.

Strategy (self-contained; shapes hardcoded for the nn_Net_14963666059852 spec):
  - Reformulate GCNConv with tables at the narrow width of each layer:
      L1 aggregate-first:  H1 = relu((dis * sum x_hat[s]) @ W1 + b1),
                           x_hat = x * dis  (table F=128)
      L2 matmul-first:     table Hh2 = (H1 @ W2) * dis  (F=256)
      L3 matmul-first:     table Hh3 = (H2 @ W3) * dis  (F=128)
      z  = dis * sum Hh3[s] + b3  (table F=128 for decode)
  - ONE AllGather per layer into a node-id-indexed big table [100000, F]
    (hits the large-transfer bandwidth plateau); gather calls window into
    it with int16 indices relative to a per-quarter base offset.
  - Self-loop contribution added densely per tile via identity matmul
    (no self edges in the edge list).
  - Nodes sharded 12500/core; edges partitioned by dst core, sorted by
    (supertile of dst, src-quarter, dst tile); each (tile, quarter)
    segment padded to a multiple of 128 slots; chunk counts commonized
    across cores so one SPMD program serves all 8 cores.
  - Per 128-edge chunk: onehot(edge -> dst slot) built on DVE (one batched
    is_equal per gather call via broadcast APs); PE matmul accumulates the
    segment-sum in PSUM.  Messages come from dma_gather (int16 idx).
  - Few big SWDGE gather calls (CAPX chunks each) to amortize the ~1us
    fixed descriptor-generation cost per call; 4 SWDGE queues round-robin.
  - Decode: eval pairs sharded 50k/core, grouped by (quarter(a),
    quarter(b)); unified call stream gathers za/zb, one batched DVE
    mul + reduce_sum per call, written straight to the logits DRAM slice.
  - Inputs shipped small (dispatch overhead scales with bytes): x_hat in
    bf16, idx/ea/eb at [16, cols] replicated to 128 partitions on device,
    dstloc in bf16, weights in bf16, biases as [1, F] rows.
"""

import os as _os_mod

import numpy as np
import ml_dtypes

import concourse.bass as bass
import concourse.bacc as bacc
import concourse.tile as tile
import concourse.mybir as mybir
from concourse import library_config
from concourse.masks import make_identity
from concourse.bass_utils import run_bass_kernel_spmd

# ---------------- problem constants (hardcoded per spec) ----------------
N = 100000
NCORES = 8
NPC = N // NCORES          # 12500 nodes per core
P = 128
TPC = (NPC + P - 1) // P   # 98 tiles per core (last has 84 rows)
LAST_ROWS = NPC - (TPC - 1) * P  # 84
G = 4                      # tiles per supertile
NST = (TPC + G - 1) // G   # 25 supertiles
NQ = 4                     # quarter windows of the big table
QR = N // NQ               # 25000 rows per window (int16-addressable)
IN_C, HID_C, OUT_C = 128, 256, 128
NEVAL_PC = 50000           # eval pairs per core

CAPX = int(_os_mod.environ.get("CAPX", "8"))        # chunks per agg call
CAP_EV = int(_os_mod.environ.get("CAP_EV", "8"))    # chunks per decode call
NQUEUES = int(_os_mod.environ.get("NQUEUES", "4"))   # SWDGE queues
DMA_SCRATCH = int(_os_mod.environ.get("DMA_SCRATCH", "16384"))
ONEHOT_SCALAR = _os_mod.environ.get("ONEHOT_SCALAR", "1") == "1"
TRANS_F32 = _os_mod.environ.get("TRANS_F32", "") == "1"

F32 = mybir.dt.float32
I16 = mybir.dt.int16
GDT = mybir.dt.bfloat16
NPBF = ml_dtypes.bfloat16


# ======================================================================
# host-side preprocessing
# ======================================================================

def _ceil_div(a, b):
    return (a + b - 1) // b


def _pack_idx16(slot_vals, calls, ncols):
    """Pack per-slot int16 indices into the [16, ncols] 16-wrapped layout.

    calls: list of (slot0, nslots, col0). Within a call, slot i ->
    [i % 16, col0 + i // 16]. Replication to 128 partitions happens
    on-device.
    """
    arr = np.zeros((16, ncols), np.int16)
    for slot0, nslots, col0 in calls:
        s = slot_vals[slot0 : slot0 + nslots]
        arr[:, col0 : col0 + nslots // 16] = s.reshape(-1, 16).T
    return arr


def _balance_perm(deg_counts):
    """old->new node permutation: serpentine-deal nodes (sorted by in-degree
    desc) across the 784 global tiles so every tile has near-equal edge load.
    Global tile g = c * TPC + t gets nodes new_id in
    [c * NPC + t * P, c * NPC + t * P + size), size = 128 (84 for t = 97)."""
    nbins = NCORES * TPC
    sizes = np.full(nbins, P, np.int64)
    sizes[TPC - 1 :: TPC] = LAST_ROWS
    c_of = np.arange(nbins) // TPC
    t_of = np.arange(nbins) % TPC
    base = c_of * NPC + t_of * P

    order = np.argsort(-deg_counts, kind="stable")  # old ids, heavy first
    old2new = np.empty(N, np.int64)
    fill = np.zeros(nbins, np.int64)
    pos = 0
    rnd = 0
    while pos < N:
        bins = np.arange(nbins) if rnd % 2 == 0 else np.arange(nbins)[::-1]
        avail = bins[fill[bins] < sizes[bins]]
        take = min(len(avail), N - pos)
        avail = avail[:take]
        old2new[order[pos : pos + take]] = base[avail] + fill[avail]
        fill[avail] += 1
        pos += take
        rnd += 1
    return old2new


def _split_calls(cc, cap):
    """Split cc chunks into near-equal parts of <= cap."""
    if cc == 0:
        return []
    nparts = _ceil_div(cc, cap)
    base = cc // nparts
    rem = cc % nparts
    return [base + (1 if i < rem else 0) for i in range(nparts)]


def _preprocess(x, edge_index, pos_edge_index, neg_edge_index):
    src0 = np.asarray(edge_index[0], dtype=np.int64)
    dst0 = np.asarray(edge_index[1], dtype=np.int64)

    deg_counts = np.bincount(dst0, minlength=N)
    deg = deg_counts.astype(np.float32) + np.float32(1.0)
    dis0 = (np.float32(1.0) / np.sqrt(deg)).astype(np.float32)

    # node permutation balancing per-tile edge counts across cores
    old2new = _balance_perm(deg_counts)
    new2old = np.empty(N, np.int64)
    new2old[old2new] = np.arange(N)

    src = old2new[src0]
    dst = old2new[dst0]
    x = np.asarray(x, dtype=np.float32)[new2old]
    dis = dis0[new2old]
    xhat = (x * dis[:, None]).astype(NPBF)          # [N, IN_C] bf16

    # ---- per-core segment counts (no self edges; dense self-add on device)
    core_of = dst // NPC
    tloc = (dst - core_of * NPC) // P               # 0..97
    q = src // QR                                    # 0..3 window
    row = (src - q * QR).astype(np.int16)            # 0..24999
    segkey = tloc * NQ + q                           # 0..391

    counts = np.zeros((NCORES, TPC, NQ), np.int64)
    per_core = []
    for c in range(NCORES):
        m = core_of == c
        sk = segkey[m]
        counts[c] = np.bincount(sk, minlength=TPC * NQ).reshape(TPC, NQ)
        order = np.argsort(sk, kind="stable")
        s_l = row[m][order]
        d_l = (dst[m] - c * NPC - tloc[m] * P)[order].astype(np.float32)
        seg_off = np.zeros(TPC * NQ + 1, np.int64)
        np.cumsum(counts[c].reshape(-1), out=seg_off[1:])
        per_core.append((s_l, d_l, seg_off))

    cch = _ceil_div(counts.max(axis=0), P)          # [TPC, NQ] common chunks

    # ---- common structural schedule ----
    # chunk order: st-major, then q, then tile. one seg = (t, q) chunks
    seg_chunk_off = np.zeros((TPC, NQ), np.int64)
    chunk_tile = []        # global chunk -> tile
    calls = []             # dicts: st, q, ch0, nch, col0
    ch = 0
    col = 0
    for st in range(NST):
        t_lo, t_hi = G * st, min(G * st + G, TPC)
        for qq in range(NQ):
            cc = int(cch[t_lo:t_hi, qq].sum())
            if cc == 0:
                continue
            for t in range(t_lo, t_hi):
                seg_chunk_off[t, qq] = ch + int(cch[t_lo:t, qq].sum())
            tiles_seq = np.repeat(np.arange(t_lo, t_hi), cch[t_lo:t_hi, qq])
            chunk_tile.extend(tiles_seq.tolist())
            sub0 = 0
            for n in _split_calls(cc, CAPX):
                calls.append(dict(st=st, q=qq, ch0=ch + sub0, nch=n, col0=col))
                col += n * 8  # n*128 slots / 16
                sub0 += n
            ch += cc
    totch = ch
    idxcols = col
    chunk_tile = np.array(chunk_tile, np.int64)

    first_ch = np.full(TPC, -1, np.int64)
    for k in range(totch - 1, -1, -1):
        first_ch[chunk_tile[k]] = k

    # ---- per-core slot arrays ----
    pack_calls = [(cal["ch0"] * P, cal["nch"] * P, cal["col0"]) for cal in calls]
    idx16_list, dstloc_list = [], []
    for c in range(NCORES):
        s_l, d_l, seg_off = per_core[c]
        slot_idx = np.zeros(totch * P, np.int16)
        slot_dst = np.full(totch * P, -1.0, np.float32)
        for t in range(TPC):
            for qq in range(NQ):
                n = int(counts[c, t, qq])
                if n == 0:
                    continue
                so = int(seg_chunk_off[t, qq]) * P
                o0 = int(seg_off[t * NQ + qq])
                slot_idx[so : so + n] = s_l[o0 : o0 + n]
                slot_dst[so : so + n] = d_l[o0 : o0 + n]
        idx16_list.append(_pack_idx16(slot_idx, pack_calls, idxcols))
        dstloc_list.append(
            np.ascontiguousarray(slot_dst.reshape(totch, P).T).astype(NPBF)
        )

    # ---- per-core dense inputs ----
    xh_list, disT_list = [], []
    for c in range(NCORES):
        xh_list.append(np.ascontiguousarray(xhat[c * NPC : (c + 1) * NPC]))
        dd = np.ones(TPC * P, np.float32)
        dd[:NPC] = dis[c * NPC : (c + 1) * NPC]
        disT_list.append(np.ascontiguousarray(dd.reshape(TPC, P).T))

    # ---- eval pairs ----
    ei = old2new[
        np.concatenate(
            [np.asarray(pos_edge_index, np.int64),
             np.asarray(neg_edge_index, np.int64)],
            axis=1,
        )
    ]
    ev_per_core = []
    ev_counts = np.zeros((NCORES, NQ * NQ), np.int64)
    for c in range(NCORES):
        a = ei[0, c * NEVAL_PC : (c + 1) * NEVAL_PC]
        b = ei[1, c * NEVAL_PC : (c + 1) * NEVAL_PC]
        qa = a // QR
        ra = (a - qa * QR).astype(np.int16)
        qb = b // QR
        rb = (b - qb * QR).astype(np.int16)
        g = qa * NQ + qb
        order = np.argsort(g, kind="stable")
        ev_counts[c] = np.bincount(g, minlength=NQ * NQ)
        ev_per_core.append((ra[order], rb[order], order, g[order]))

    ech = _ceil_div(ev_counts.max(axis=0), P)       # [16]
    ev_goff = np.zeros(NQ * NQ + 1, np.int64)
    np.cumsum(ech, out=ev_goff[1:])
    etotch = int(ev_goff[-1])

    ecalls = []
    ecol = 0
    for qa in range(NQ):
        for qb in range(NQ):
            g0 = int(ev_goff[qa * NQ + qb])
            g1 = int(ev_goff[qa * NQ + qb + 1])
            sub = g0
            for n in _split_calls(g1 - g0, CAP_EV):
                ecalls.append(dict(qa=qa, qb=qb, ch0=sub, nch=n, col0=ecol))
                ecol += n * 8
                sub += n
    eacols = ecol

    ea16_list, eb16_list, evmap_list = [], [], []
    apack = [(c["ch0"] * P, c["nch"] * P, c["col0"]) for c in ecalls]
    for c in range(NCORES):
        a_s, b_s, order, g_s = ev_per_core[c]
        slot_a = np.zeros(etotch * P, np.int16)
        slot_b = np.zeros(etotch * P, np.int16)
        evmap = np.full(etotch * P, -1, np.int64)
        n = len(a_s)
        cumstart = np.zeros(NQ * NQ + 1, np.int64)
        np.cumsum(ev_counts[c], out=cumstart[1:])
        pos_in_g = np.arange(n, dtype=np.int64) - cumstart[g_s]
        slots = ev_goff[g_s] * P + pos_in_g
        slot_a[slots] = a_s
        slot_b[slots] = b_s
        evmap[slots] = c * NEVAL_PC + order
        ea16_list.append(_pack_idx16(slot_a, apack, eacols))
        eb16_list.append(_pack_idx16(slot_b, apack, eacols))
        evmap_list.append(evmap)

    meta = dict(
        calls=calls, totch=totch, idxcols=idxcols,
        chunk_tile=chunk_tile, first_ch=first_ch,
        ecalls=ecalls, etotch=etotch, eacols=eacols,
    )
    percore = dict(
        idx16=idx16_list, dstloc=dstloc_list, xh=xh_list, disT=disT_list,
        ea16=ea16_list, eb16=eb16_list, evmap=evmap_list,
    )
    return meta, percore, dis


# ======================================================================
# program build
# ======================================================================

def _build_program(meta, ablate=None, reps=1, has_bias=True):
    import os as _os
    if ablate is None:
        ablate = _os.environ.get("ABLATE", "")
    totch = meta["totch"]
    idxcols = meta["idxcols"]
    etotch = meta["etotch"]
    eacols = meta["eacols"]

    nc = bacc.Bacc("TRN2", target_bir_lowering=False, debug=False,
                   num_devices=NCORES, num_swdge_queues=NQUEUES,
                   dynamic_dma_scratch_size=DMA_SCRATCH)

    xh_in = nc.dram_tensor("xh", [NPC, IN_C], GDT, kind="ExternalInput")
    disT_in = nc.dram_tensor("disT", [P, TPC], F32, kind="ExternalInput")
    idx_in = nc.dram_tensor("idx", [16, idxcols], I16, kind="ExternalInput")
    dstloc_in = nc.dram_tensor("dstloc", [P, totch], GDT, kind="ExternalInput")
    ea_in = nc.dram_tensor("ea", [16, eacols], I16, kind="ExternalInput")
    eb_in = nc.dram_tensor("eb", [16, eacols], I16, kind="ExternalInput")
    W1_in = nc.dram_tensor("W1", [IN_C, HID_C], GDT, kind="ExternalInput")
    W2_in = nc.dram_tensor("W2", [HID_C, HID_C], GDT, kind="ExternalInput")
    W3_in = nc.dram_tensor("W3", [HID_C, OUT_C], GDT, kind="ExternalInput")
    b1_in = nc.dram_tensor("b1r", [1, HID_C], GDT, kind="ExternalInput")
    b2_in = nc.dram_tensor("b2r", [1, HID_C], GDT, kind="ExternalInput")
    b3_in = nc.dram_tensor("b3r", [1, OUT_C], GDT, kind="ExternalInput")

    logits_out = nc.dram_tensor("logits", [P, etotch], F32, kind="ExternalOutput")

    calls = meta["calls"]
    chunk_tile = meta["chunk_tile"]
    first_ch = meta["first_ch"]

    def rows_of(t):
        return LAST_ROWS if t == TPC - 1 else P

    with tile.TileContext(nc) as tc:
        with (
            tc.tile_pool(name="const", bufs=1) as cst,
            tc.tile_pool(name="sb", bufs=2) as sb,
            tc.tile_pool(name="ps", bufs=2, space="PSUM") as ps,
            tc.tile_pool(name="dram", bufs=1, space="DRAM") as dr,
        ):
            nc.gpsimd.load_library(library_config.mlp)

            # ---------------- constants ----------------
            ident = cst.tile([P, P], GDT)
            make_identity(nc, ident[:])
            TDT = F32 if TRANS_F32 else GDT
            if TRANS_F32:
                identT = cst.tile([P, P], F32)
                make_identity(nc, identT[:])
            else:
                identT = ident
            iota_t = cst.tile([P, P], GDT)
            nc.gpsimd.iota(iota_t[:], [[1, P]], channel_multiplier=0,
                           allow_small_or_imprecise_dtypes=True)
            disT_t = cst.tile([P, TPC], F32)
            nc.sync.dma_start(out=disT_t[:], in_=disT_in[:, :])
            idx_t = cst.tile([P, idxcols], I16)
            for k in range(8):
                nc.sync.dma_start(out=idx_t[16 * k : 16 * (k + 1), :],
                                  in_=idx_in[:, :])
            dstloc_t = cst.tile([P, totch], GDT)
            nc.sync.dma_start(out=dstloc_t[:], in_=dstloc_in[:, :])
            if ONEHOT_SCALAR:
                dstloc_f = cst.tile([P, totch], F32)
                nc.vector.tensor_copy(out=dstloc_f[:], in_=dstloc_t[:])
            W1_t = cst.tile([IN_C, HID_C], GDT)
            nc.sync.dma_start(out=W1_t[:], in_=W1_in[:, :])
            W2a_t = cst.tile([P, HID_C], GDT)
            nc.sync.dma_start(out=W2a_t[:], in_=W2_in[0:P, :])
            W2b_t = cst.tile([P, HID_C], GDT)
            nc.sync.dma_start(out=W2b_t[:], in_=W2_in[P : 2 * P, :])
            W3a_t = cst.tile([P, OUT_C], GDT)
            nc.sync.dma_start(out=W3a_t[:], in_=W3_in[0:P, :])
            W3b_t = cst.tile([P, OUT_C], GDT)
            nc.sync.dma_start(out=W3b_t[:], in_=W3_in[P : 2 * P, :])

            ones1 = cst.tile([1, P], GDT)
            nc.vector.memset(ones1[:], 1.0)
            b1row = cst.tile([1, HID_C], GDT)
            nc.sync.dma_start(out=b1row[:], in_=b1_in[:, :])
            # broadcast tiles for b2, b3 (via ones^T @ b row)
            if has_bias:
                b2row = cst.tile([1, HID_C], GDT)
                nc.sync.dma_start(out=b2row[:], in_=b2_in[:, :])
                b3row = cst.tile([1, OUT_C], GDT)
                nc.sync.dma_start(out=b3row[:], in_=b3_in[:, :])
                b2bc = cst.tile([P, HID_C], F32)
                bp = ps.tile([P, HID_C], F32, space="PSUM", tag="hp", bufs=2,
                             name="bp2")
                nc.tensor.matmul(out=bp[:], lhsT=ones1[:], rhs=b2row[:],
                                 start=True, stop=True)
                nc.scalar.activation(out=b2bc[:], in_=bp[:],
                                     func=mybir.ActivationFunctionType.Copy)
                b3bc = cst.tile([P, OUT_C], F32)
                bp3 = ps.tile([P, OUT_C], F32, space="PSUM", tag="hp", bufs=2,
                              name="bp3")
                nc.tensor.matmul(out=bp3[:], lhsT=ones1[:], rhs=b3row[:],
                                 start=True, stop=True)
                nc.scalar.activation(out=b3bc[:], in_=bp3[:],
                                     func=mybir.ActivationFunctionType.Copy)

            for _rep in range(reps):
                # ---------------- DRAM tables ----------------
                xsh = dr.tile([NPC, IN_C], GDT, name=f"xsh{_rep}")
                h2sh = dr.tile([NPC, HID_C], GDT, name=f"h2sh{_rep}")
                h3sh = dr.tile([NPC, OUT_C], GDT, name=f"h3sh{_rep}")
                zsh = dr.tile([NPC, OUT_C], GDT, name=f"zsh{_rep}")
                xbig = dr.tile([N, IN_C], GDT, addr_space="Shared",
                               name=f"xbig{_rep}")
                h2big = dr.tile([N, HID_C], GDT, addr_space="Shared",
                                name=f"h2big{_rep}")
                h3big = dr.tile([N, OUT_C], GDT, addr_space="Shared",
                                name=f"h3big{_rep}")
                zbig = dr.tile([N, OUT_C], GDT, addr_space="Shared",
                               name=f"zbig{_rep}")

                def ag(sh_ap, big):
                    if "nocoll" in ablate:
                        return
                    nc.gpsimd.collective_compute(
                        "AllGather", mybir.AluOpType.bypass,
                        ins=[sh_ap.opt()],
                        outs=[big[0:N, :].opt()],
                        replica_groups=[list(range(NCORES))],
                    )

                nc.sync.dma_start(out=xsh[0:NPC, :], in_=xh_in[0:NPC, :])
                ag(xsh[0:NPC, :], xbig)

                # ---------------- aggregation sweep ----------------
                def sweep(table, f_l, selfsrc, tail, sid):
                    no_g = "nogather" in ablate
                    no_oh = "nooh" in ablate
                    no_mm = "nomm" in ablate
                    no_tail = "notail" in ablate
                    dummy_msg = dummy_oh = None
                    if no_g:
                        dummy_msg = sb.tile([P, CAPX, f_l], GDT, tag="msg",
                                            bufs=1, name=f"dmsg{sid}")
                        nc.gpsimd.dma_gather(
                            out_ap=dummy_msg[:],
                            in_ap=table[0:QR, :],
                            idxs_ap=idx_t[:, 0 : CAPX * 8],
                            num_idxs=CAPX * P, num_idxs_reg=CAPX * P,
                            elem_size=f_l, queue_num=0,
                        )
                    if no_oh:
                        dummy_oh = sb.tile([P, CAPX, P], GDT, tag="oh",
                                           bufs=1, name=f"doh{sid}")
                        nc.vector.tensor_tensor(
                            out=dummy_oh[:],
                            in0=iota_t[:].unsqueeze(1).broadcast_to(
                                [P, CAPX, P]),
                            in1=dstloc_t[:, 0:CAPX].unsqueeze(
                                2).broadcast_to([P, CAPX, P]),
                            op=mybir.AluOpType.is_equal,
                        )
                    ci = 0
                    for st in range(NST):
                        t_lo, t_hi = G * st, min(G * st + G, TPC)
                        aggp = {}
                        for t in range(t_lo, t_hi):
                            aggp[t] = ps.tile(
                                [P, f_l], F32, space="PSUM", tag="agg",
                                bufs=G, name=f"agg{sid}_{st}_{t}",
                            )
                        while ci < len(calls) and calls[ci]["st"] == st:
                            cal = calls[ci]
                            nch = cal["nch"]
                            ch0 = cal["ch0"]
                            if no_g:
                                msg = dummy_msg
                            else:
                                msg = sb.tile([P, nch, f_l], GDT, tag="msg",
                                              bufs=12, name=f"msg{sid}_{ci}")
                                nc.gpsimd.dma_gather(
                                    out_ap=msg[:],
                                    in_ap=table[cal["q"] * QR : (cal["q"] + 1) * QR, :],
                                    idxs_ap=idx_t[:, cal["col0"] : cal["col0"] + nch * 8],
                                    num_idxs=nch * P,
                                    num_idxs_reg=nch * P,
                                    elem_size=f_l,
                                    queue_num=ci % NQUEUES,
                                )
                            if no_oh:
                                oh = dummy_oh
                            else:
                                oh = sb.tile([P, nch, P], GDT, tag="oh", bufs=12,
                                             name=f"oh{sid}_{ci}")
                                if ONEHOT_SCALAR:
                                    for j in range(nch):
                                        nc.vector.tensor_scalar(
                                            out=oh[:, j, :], in0=iota_t[:],
                                            scalar1=dstloc_f[:, ch0 + j : ch0 + j + 1],
                                            scalar2=None,
                                            op0=mybir.AluOpType.is_equal,
                                        )
                                else:
                                    nc.vector.tensor_tensor(
                                        out=oh[:],
                                        in0=iota_t[:].unsqueeze(1).broadcast_to(
                                            [P, nch, P]),
                                        in1=dstloc_t[:, ch0 : ch0 + nch].unsqueeze(
                                            2).broadcast_to([P, nch, P]),
                                        op=mybir.AluOpType.is_equal,
                                    )
                            if not no_mm:
                                for j in range(nch):
                                    k = ch0 + j
                                    t = int(chunk_tile[k])
                                    nc.tensor.matmul(
                                        out=aggp[t][:], lhsT=oh[:, j, :],
                                        rhs=msg[:, j, :],
                                        start=(k == int(first_ch[t])),
                                        stop=False,
                                    )
                            ci += 1
                        for t in range(t_lo, t_hi):
                            r = rows_of(t)
                            selft = sb.tile([P, f_l], GDT, tag="selft",
                                            bufs=4, name=f"st{sid}_{t}")
                            if r < P:
                                nc.vector.memset(selft[:], 0.0)
                            nc.sync.dma_start(
                                out=selft[:r, :],
                                in_=selfsrc[t * P : t * P + r, :],
                            )
                            nc.tensor.matmul(
                                out=aggp[t][:], lhsT=ident[:], rhs=selft[:],
                                start=(no_mm or int(first_ch[t]) < 0),
                                stop=True,
                            )
                            if not no_tail:
                                tail(t, aggp[t], r)

                # ---------------- layer tails ----------------
                def tail1(t, aggp, r):
                    dis_col = disT_t[:, t : t + 1]
                    xp = sb.tile([P, IN_C], TDT, tag="xp", bufs=3,
                                 name=f"xp{_rep}_{t}")
                    nc.scalar.activation(
                        out=xp[:], in_=aggp[:],
                        func=mybir.ActivationFunctionType.Copy,
                        scale=dis_col)
                    tp = ps.tile([P, IN_C], TDT, space="PSUM", tag="tp",
                                 bufs=2, name=f"tp1{_rep}_{t}")
                    nc.tensor.transpose(out=tp[:], in_=xp[:], identity=identT[:])
                    xpT = sb.tile([P, IN_C], GDT, tag="xpT", bufs=3,
                                  name=f"xpT{_rep}_{t}")
                    nc.vector.tensor_copy(out=xpT[:], in_=tp[:])
                    h1p = ps.tile([P, HID_C], F32, space="PSUM", tag="hp",
                                  bufs=2, name=f"h1p{_rep}_{t}")
                    nc.tensor.matmul(out=h1p[:], lhsT=xpT[:], rhs=W1_t[:],
                                     start=True, stop=not has_bias)
                    if has_bias:
                        nc.tensor.matmul(out=h1p[:], lhsT=ones1[:],
                                         rhs=b1row[:], start=False, stop=True)
                    H1 = sb.tile([P, HID_C], TDT, tag="H", bufs=3,
                                 name=f"H1{_rep}_{t}")
                    nc.scalar.activation(
                        out=H1[:], in_=h1p[:],
                        func=mybir.ActivationFunctionType.Relu)
                    hT = sb.tile([P, HID_C], GDT, tag="hT", bufs=3,
                                 name=f"h1T{_rep}_{t}")
                    for bb in range(2):
                        tp2 = ps.tile([P, P], TDT, space="PSUM", tag="tp",
                                      bufs=2, name=f"tp1b{_rep}_{t}_{bb}")
                        nc.tensor.transpose(
                            out=tp2[:], in_=H1[:, bb * P : (bb + 1) * P],
                            identity=identT[:])
                        nc.vector.tensor_copy(
                            out=hT[:, bb * P : (bb + 1) * P], in_=tp2[:])
                    h2p = ps.tile([P, HID_C], F32, space="PSUM", tag="hp",
                                  bufs=2, name=f"h2p{_rep}_{t}")
                    nc.tensor.matmul(out=h2p[:], lhsT=hT[:, 0:P], rhs=W2a_t[:],
                                     start=True, stop=False)
                    nc.tensor.matmul(out=h2p[:], lhsT=hT[:, P : 2 * P],
                                     rhs=W2b_t[:], start=False, stop=True)
                    hh2 = sb.tile([P, HID_C], GDT, tag="hh", bufs=4,
                                  name=f"hh2{_rep}_{t}")
                    nc.scalar.activation(
                        out=hh2[:], in_=h2p[:],
                        func=mybir.ActivationFunctionType.Copy,
                        scale=dis_col)
                    nc.sync.dma_start(out=h2sh[t * P : t * P + r, :],
                                      in_=hh2[:r, :])

                def tail2(t, aggp, r):
                    dis_col = disT_t[:, t : t + 1]
                    H2 = sb.tile([P, HID_C], TDT, tag="H", bufs=3,
                                 name=f"H2{_rep}_{t}")
                    if has_bias:
                        xr = sb.tile([P, HID_C], F32, tag="xr", bufs=3,
                                     name=f"xr{_rep}_{t}")
                        nc.vector.scalar_tensor_tensor(
                            out=xr[:], in0=aggp[:], scalar=dis_col,
                            in1=b2bc[:], op0=mybir.AluOpType.mult,
                            op1=mybir.AluOpType.add)
                        nc.scalar.activation(
                            out=H2[:], in_=xr[:],
                            func=mybir.ActivationFunctionType.Relu)
                    else:
                        nc.scalar.activation(
                            out=H2[:], in_=aggp[:],
                            func=mybir.ActivationFunctionType.Relu,
                            scale=dis_col)
                    hT = sb.tile([P, HID_C], GDT, tag="hT", bufs=3,
                                 name=f"h2T{_rep}_{t}")
                    for bb in range(2):
                        tp2 = ps.tile([P, P], TDT, space="PSUM", tag="tp",
                                      bufs=2, name=f"tp2b{_rep}_{t}_{bb}")
                        nc.tensor.transpose(
                            out=tp2[:], in_=H2[:, bb * P : (bb + 1) * P],
                            identity=identT[:])
                        nc.vector.tensor_copy(
                            out=hT[:, bb * P : (bb + 1) * P], in_=tp2[:])
                    h3p = ps.tile([P, OUT_C], F32, space="PSUM", tag="hp",
                                  bufs=2, name=f"h3p{_rep}_{t}")
                    nc.tensor.matmul(out=h3p[:], lhsT=hT[:, 0:P], rhs=W3a_t[:],
                                     start=True, stop=False)
                    nc.tensor.matmul(out=h3p[:], lhsT=hT[:, P : 2 * P],
                                     rhs=W3b_t[:], start=False, stop=True)
                    hh3 = sb.tile([P, OUT_C], GDT, tag="hh", bufs=4,
                                  name=f"hh3{_rep}_{t}")
                    nc.scalar.activation(
                        out=hh3[:], in_=h3p[:],
                        func=mybir.ActivationFunctionType.Copy,
                        scale=dis_col)
                    nc.sync.dma_start(out=h3sh[t * P : t * P + r, :],
                                      in_=hh3[:r, :])

                def tail3(t, aggp, r):
                    dis_col = disT_t[:, t : t + 1]
                    zt = sb.tile([P, OUT_C], GDT, tag="hh", bufs=4,
                                 name=f"zt{_rep}_{t}")
                    if has_bias:
                        nc.vector.scalar_tensor_tensor(
                            out=zt[:], in0=aggp[:], scalar=dis_col,
                            in1=b3bc[:], op0=mybir.AluOpType.mult,
                            op1=mybir.AluOpType.add)
                    else:
                        nc.scalar.activation(
                            out=zt[:], in_=aggp[:],
                            func=mybir.ActivationFunctionType.Copy,
                            scale=dis_col)
                    nc.sync.dma_start(out=zsh[t * P : t * P + r, :],
                                      in_=zt[:r, :])

                if ablate != "coll_only":
                    sweep(xbig, IN_C, xh_in, tail1, f"s1_{_rep}")
                ag(h2sh[0:NPC, :], h2big)
                if ablate != "coll_only":
                    sweep(h2big, HID_C, h2sh, tail2, f"s2_{_rep}")
                ag(h3sh[0:NPC, :], h3big)
                if ablate != "coll_only":
                    sweep(h3big, OUT_C, h3sh, tail3, f"s3_{_rep}")
                ag(zsh[0:NPC, :], zbig)

                # ---------------- decode ----------------
                if "nodecode" not in ablate and ablate != "coll_only":
                    if _rep == 0:
                        ea_t = cst.tile([P, eacols], I16, name="ea_t")
                        eb_t = cst.tile([P, eacols], I16, name="eb_t")
                        for k in range(8):
                            nc.sync.dma_start(
                                out=ea_t[16 * k : 16 * (k + 1), :],
                                in_=ea_in[:, :])
                            nc.sync.dma_start(
                                out=eb_t[16 * k : 16 * (k + 1), :],
                                in_=eb_in[:, :])
                    for ei, cal in enumerate(meta["ecalls"]):
                        nch = cal["nch"]
                        cslice = slice(cal["col0"], cal["col0"] + nch * 8)
                        za = sb.tile([P, nch, OUT_C], GDT, tag="za", bufs=2,
                                     name=f"za{_rep}_{ei}")
                        nc.gpsimd.dma_gather(
                            out_ap=za[:],
                            in_ap=zbig[cal["qa"] * QR : (cal["qa"] + 1) * QR, :],
                            idxs_ap=ea_t[:, cslice],
                            num_idxs=nch * P, num_idxs_reg=nch * P,
                            elem_size=OUT_C,
                            queue_num=(2 * ei) % NQUEUES,
                        )
                        zb = sb.tile([P, nch, OUT_C], GDT, tag="zb", bufs=2,
                                     name=f"zb{_rep}_{ei}")
                        nc.gpsimd.dma_gather(
                            out_ap=zb[:],
                            in_ap=zbig[cal["qb"] * QR : (cal["qb"] + 1) * QR, :],
                            idxs_ap=eb_t[:, cslice],
                            num_idxs=nch * P, num_idxs_reg=nch * P,
                            elem_size=OUT_C,
                            queue_num=(2 * ei + 1) % NQUEUES,
                        )
                        prod = sb.tile([P, nch, OUT_C], F32, tag="prod",
                                       bufs=2, name=f"prod{_rep}_{ei}")
                        nc.vector.tensor_mul(out=prod[:], in0=za[:], in1=zb[:])
                        lseg = sb.tile([P, nch], F32, tag="lseg", bufs=2,
                                       name=f"lseg{_rep}_{ei}")
                        nc.vector.reduce_sum(out=lseg[:], in_=prod[:],
                                             axis=mybir.AxisListType.X)
                        nc.sync.dma_start(
                            out=logits_out[:, cal["ch0"] : cal["ch0"] + nch],
                            in_=lseg[:],
                        )

    nc.compile()
    return nc


# ======================================================================
# entry point helpers
# ======================================================================

def _make_in_maps(inputs, meta, percore):
    W1 = np.asarray(inputs["W1"], np.float32).astype(NPBF)
    W2 = np.asarray(inputs["W2"], np.float32).astype(NPBF)
    W3 = np.asarray(inputs["W3"], np.float32).astype(NPBF)
    b1 = np.asarray(inputs["b1"], np.float32).astype(NPBF)[None, :]
    b2 = np.asarray(inputs["b2"], np.float32).astype(NPBF)[None, :]
    b3 = np.asarray(inputs["b3"], np.float32).astype(NPBF)[None, :]
    in_maps = []
    for c in range(NCORES):
        in_maps.append(
            dict(
                xh=percore["xh"][c], disT=percore["disT"][c],
                idx=percore["idx16"][c], dstloc=percore["dstloc"][c],
                ea=percore["ea16"][c], eb=percore["eb16"][c],
                W1=W1, W2=W2, W3=W3, b1r=b1, b2r=b2, b3r=b3,
            )
        )
    return in_maps


def _has_bias(inputs):
    return bool(
        np.abs(np.asarray(inputs["b1"])).max() > 0
        or np.abs(np.asarray(inputs["b2"])).max() > 0
        or np.abs(np.asarray(inputs["b3"])).max() > 0
    )


def _unpack_logits(raw_list_or_arr, meta, percore):
    etotch = meta["etotch"]
    logits = np.zeros(NCORES * NEVAL_PC, np.float32)
    for c in range(NCORES):
        out = np.asarray(raw_list_or_arr[c]).reshape(P, etotch)
        vals = out.T.reshape(-1)   # slot s = col*128+p ordering
        evmap = percore["evmap"][c]
        valid = evmap >= 0
        logits[evmap[valid]] = vals[valid]
    return logits


def _run(inputs):
    x = np.asarray(inputs["x"], np.float32)
    meta, percore, _dis = _preprocess(
        x, inputs["edge_index"], inputs["pos_edge_index"],
        inputs["neg_edge_index"],
    )
    nc = _build_program(meta, has_bias=_has_bias(inputs))
    in_maps = _make_in_maps(inputs, meta, percore)
    res = run_bass_kernel_spmd(
        nc, in_maps, core_ids=list(range(NCORES)), trace=False
    )
    raw = [res.results[c]["logits"] for c in range(NCORES)]
    return _unpack_logits(raw, meta, percore), res


def kernel(**inputs):
    logits, _ = _run(inputs)
    return logits


# ======================================================================
# wall-clock benchmarking (no NTFF hook in this container)
# ======================================================================

def _make_sharded_exec(nc, in_maps, donate=False):
    """Mimic bass2jax.run_bass_via_pjrt's multi-core path but keep the jitted
    callable so repeat executions can be timed with device-resident inputs."""
    import jax
    from jax.sharding import Mesh, PartitionSpec
    from jax.experimental.shard_map import shard_map
    import concourse.mybir as mb
    from concourse.bass2jax import (
        _bass_exec_p, install_neuronx_cc_hook, partition_id_tensor,
    )

    install_neuronx_cc_hook()
    partition_name = (
        nc.partition_id_tensor.name if nc.partition_id_tensor else None
    )
    in_names, out_names, out_avals, zero_outs = [], [], [], []
    for alloc in nc.m.functions[0].allocations:
        if not isinstance(alloc, mb.MemoryLocationSet):
            continue
        name = alloc.memorylocations[0].name
        if alloc.kind == "ExternalInput":
            if name != partition_name:
                in_names.append(name)
        elif alloc.kind == "ExternalOutput":
            out_names.append(name)
            shape = tuple(alloc.tensor_shape)
            dtype = mb.dt.np(alloc.dtype)
            out_avals.append(jax.core.ShapedArray(shape, dtype))
            zero_outs.append(np.zeros(shape, dtype))
    n_params = len(in_names)
    n_outs = len(out_avals)
    in_names.extend(out_names)
    if partition_name is not None:
        in_names.append(partition_name)

    def _body(*args):
        operands = list(args)
        if partition_name is not None:
            operands.append(partition_id_tensor())
        return tuple(_bass_exec_p.bind(
            *operands, out_avals=tuple(out_avals), in_names=tuple(in_names),
            out_names=tuple(out_names), lowering_input_output_aliases=(),
            sim_require_finite=True, sim_require_nnan=True, nc=nc,
        ))

    devices = jax.devices()[:NCORES]
    mesh = Mesh(np.asarray(devices), ("core",))
    in_specs = (PartitionSpec("core"),) * (n_params + n_outs)
    out_specs = (PartitionSpec("core"),) * len(out_names)
    sharded = jax.jit(
        shard_map(_body, mesh=mesh, in_specs=in_specs, out_specs=out_specs,
                  check_rep=False),
        donate_argnums=tuple(range(n_params, n_params + n_outs)) if donate else (),
        keep_unused=True,
    )
    per_core = [[np.asarray(m[name]) for name in in_names[:n_params]]
                for m in in_maps]
    concat_in = [
        np.concatenate([per_core[c][i] for c in range(NCORES)], axis=0)
        for i in range(n_params)
    ]
    concat_zeros = [
        np.zeros((NCORES * z.shape[0], *z.shape[1:]), z.dtype)
        for z in zero_outs
    ]
    dev_in = [jax.device_put(a) for a in concat_in]
    dev_zero = [jax.device_put(z) for z in concat_zeros]
    return sharded, dev_in, dev_zero, out_names, out_avals


def bench(inputs, iters=5, ablate="", prep=None, do_baseline=True, reps=1):
    """Run + time. Returns (logits, per_iter_seconds_list, baseline_seconds)."""
    import time as _time
    import jax

    if prep is None:
        x = np.asarray(inputs["x"], np.float32)
        prep = _preprocess(
            x, inputs["edge_index"], inputs["pos_edge_index"],
            inputs["neg_edge_index"],
        )
    meta, percore, _dis = prep
    nc = _build_program(meta, ablate=ablate, reps=reps,
                        has_bias=_has_bias(inputs))
    in_maps = _make_in_maps(inputs, meta, percore)
    fn, dev_in, dev_zero, out_names, out_avals = _make_sharded_exec(nc, in_maps)
    outs = fn(*dev_in, *dev_zero)
    jax.block_until_ready(outs)
    times = []
    for _ in range(iters):
        t0 = _time.perf_counter()
        outs = fn(*dev_in, *dev_zero)
        jax.block_until_ready(outs)
        times.append(_time.perf_counter() - t0)

    li = out_names.index("logits")
    etotch = meta["etotch"]
    lo = np.asarray(outs[li]).reshape(NCORES, P, etotch)
    logits = _unpack_logits(lo, meta, percore)

    bl = _baseline_time(iters) if do_baseline else [0.0]
    return logits, times, bl


def _baseline_time(iters=5):
    import time as _time
    import jax

    nc = bacc.Bacc("TRN2", target_bir_lowering=False, debug=False,
                   num_devices=NCORES)
    a_in = nc.dram_tensor("a", [P, P], F32, kind="ExternalInput")
    o_out = nc.dram_tensor("o", [P, P], F32, kind="ExternalOutput")
    with tile.TileContext(nc) as tc:
        with tc.tile_pool(name="sb", bufs=1) as sb:
            t = sb.tile([P, P], F32)
            nc.sync.dma_start(out=t[:], in_=a_in[:, :])
            nc.sync.dma_start(out=o_out[:, :], in_=t[:])
    nc.compile()
    in_maps = [{"a": np.zeros((P, P), np.float32)} for _ in range(NCORES)]
    fn, dev_in, dev_zero, _, _ = _make_sharded_exec(nc, in_maps)
    outs = fn(*dev_in, *dev_zero)
    jax.block_until_ready(outs)
    times = []
    for _ in range(iters):
        t0 = _time.perf_counter()
        outs = fn(*dev_in, *dev_zero)
        jax.block_until_ready(outs)
        times.append(_time.perf_counter() - t0)
    return times
